# revision 20
# baseline (speedup 1.0000x reference)
"""Binarized ResNet Bottleneck block (dense_cnn) on 8 TRN2 NeuronCores.

Math: with inference BN folded to z*s + c (s > 0), binarize(htanh(bn(z))) ==
sign(z + c/s).  All conv weights binarize to +-1, so every conv is a GEMM with
pixels moving on the PE:

  y1 = sign(conv1(x)  + c1/s1)                     # 1x1, 256->128
  y2 = sign(conv2(y1) + c2/s2)                     # 3x3 stride 2, pad 1
  out = sign(q*conv3(y2) + convsc(x) + cc)         # q = s3/ss, cc=(c3+cs)/ss

x is split as x = hi + lo1/SIG1 + lo2/SIG2 with hi = fp16(x) and lo_k =
e4m3(r_k * SIG_k); the matching conv weights are +-S, +-S/SIG1, +-S/SIG2 (all
exact powers of two in their dtype), so each pass accumulates S*(w . r_k) into
the same f32 PSUM (residual ~2^-19 relative -> 0 output sign flips vs f32).
The fp8 passes pair the two 128-channel k-tiles into one DoubleRow matmul
(0.5 cy/col).  conv2 runs entirely in fp8: its 9 taps become 4 DoubleRow
pairs with constant address deltas + 1 pair whose second slot reads a
zeroed scratch block.  conv3 is a plain fp8 matmul with +-1 weights; its
per-channel scale q moves to a ScalarE Identity activation (scale/bias are
per-partition APs).

Sign ops are spread over two engines: ScalarE Sign gives +-1, VectorE
(is_ge, subtract 0.5) gives +-0.5 -- both exact in fp8, and the downstream
threshold columns absorb the per-sample encoding.  The final output is the
VectorE comparison (-sc <= q*conv3 + cc), i.e. {1,0}; the host maps it to
+-1.  PSUM: stage1/2 use [128,1024] two-bank pair tiles (one elementwise op
covers two matmul groups), stage3 double-buffers conv3/shortcut banks.
"""

import numpy as np
import ml_dtypes

import concourse.bass as bass
import concourse.tile as tile
from concourse import bacc, mybir
from concourse.bass_utils import run_bass_kernel_spmd

F16 = mybir.dt.float16
F32 = mybir.dt.float32
E4 = mybir.dt.float8e4
AF = mybir.ActivationFunctionType
ALU = mybir.AluOpType
DR = mybir.MatmulPerfMode.DoubleRow
E4NP = ml_dtypes.float8_e4m3  # TRN e4m3 is IEEE-style: max normal 240, exp 0xF = inf/nan

EPS = 1e-5
NB, CIN, H, W = 32, 256, 56, 56
PLANES, OUTP = 128, 512
NCORES = 8
NPC = NB // NCORES              # samples per core
HW1 = H * W                     # 3136
WP = W + 2                      # 58 padded row length
YSZ = WP * WP                   # 3364
HO = WO = 28
PT = 392                        # pixel tile (stage1: 7 rows of 56; stage3: 14x28)

S = 512.0                       # global PSUM scale
SIGS = (2.0 ** 16,)             # e4m3 residual part scales (scaled parts stay < 240)
K = len(SIGS)

# stage1 per-sample sign engine: True -> ScalarE Sign (+-1), False -> VectorE (+-0.5)
ST1_ACT = (True, False, False, False)

# per-sample input DMA pixel chunks
CHUNKS = {0: [(0, 784), (784, 784), (1568, 784), (2352, 784)]}
for _n in range(1, NPC):
    CHUNKS[_n] = [(0, 1568), (1568, 1568)]

# wts16 [128, 1280] fp16 columns: [0:256] conv1-hi (kt-major), [256:1280]
# shortcut-hi as [kt][oc][128].
_W16C = 1280
# wts8 columns: conv1-lo parts K x [kt][128], conv2 3 pairs x [2][128] then
# 3 singles x [128], conv3 [oc][128], shortcut-lo K x [kt][oc][128]
_WC2 = K * 256
_W3C = _WC2 + 1152
_WSC = _W3C + 512
_W8C = _WSC + K * 1024

# bias [128, 16] f32 columns
_B1A, _B1V, _B2F, _B2H, _QV, _CC, _HALF = 0, 1, 2, 3, 4, 8, 12

# conv2 tap pairs: (slot0 tap, slot1 tap or None, delta builder)
_PAIRS = [((0, 0), (0, 2)), ((1, 0), (1, 2)), ((2, 0), (2, 2)),
          ((0, 1), (2, 1)), ((1, 1), None)]


def build_bass():
    nc = bacc.Bacc("TRN2", target_bir_lowering=False, debug=False)
    nhi = NPC * 2 * 128 * HW1
    xhi_d = nc.dram_tensor("xhi", [nhi], F16, kind="ExternalInput")
    xlo_d = nc.dram_tensor("xlo", [K * nhi], E4, kind="ExternalInput")
    w16_d = nc.dram_tensor("w16", [128, _W16C], F16, kind="ExternalInput")
    w8_d = nc.dram_tensor("w8", [128, _W8C], E4, kind="ExternalInput")
    bias_d = nc.dram_tensor("bias", [128, 16], F32, kind="ExternalInput")
    out_d = nc.dram_tensor("out", [NPC * 128, HW1], E4, kind="ExternalOutput")
    warm_d = nc.dram_tensor("warm", [128, 8], F32, kind="ExternalOutput")

    with tile.TileContext(nc) as tc:
        import contextlib

        with contextlib.ExitStack() as ctx:
            const = ctx.enter_context(tc.tile_pool(name="const", bufs=1))
            xpool = ctx.enter_context(tc.tile_pool(name="x", bufs=1))
            ypool = ctx.enter_context(tc.tile_pool(name="y", bufs=1))
            opool = ctx.enter_context(tc.tile_pool(name="o", bufs=1))
            tpool = ctx.enter_context(tc.tile_pool(name="t", bufs=6))
            p1pool = ctx.enter_context(tc.tile_pool(name="p1", bufs=2, space="PSUM"))
            p3a = ctx.enter_context(tc.tile_pool(name="p3a", bufs=2, space="PSUM"))
            p3b = ctx.enter_context(tc.tile_pool(name="p3b", bufs=2, space="PSUM"))

            w16 = const.tile([128, _W16C], F16, tag="w16")
            # conv1-hi first so the first matmul isn't gated on the rest
            nc.scalar.dma_start(w16[:, 0:256], w16_d.ap()[:, 0:256])
            nc.scalar.dma_start(w16[:, 256:_W16C], w16_d.ap()[:, 256:_W16C])
            w8 = const.tile([128, _W8C], E4, tag="w8")
            nc.scalar.dma_start(w8[:, 0:_WC2], w8_d.ap()[:, 0:_WC2])
            nc.scalar.dma_start(w8[:, _WC2:_W8C], w8_d.ap()[:, _WC2:_W8C])
            bias = const.tile([128, 16], F32, tag="bias")
            nc.scalar.dma_start(bias[:], bias_d.ap())

            def w16s(col):
                return w16[:, col : col + 128]

            def w8dr(col):  # [128, 2, 128] DoubleRow stationary view
                return bass.AP(tensor=w8[:].tensor, offset=col,
                               ap=[w8[:].ap[0], [128, 2], [1, 128]])

            def w8s(col):
                return w8[:, col : col + 128]

            # PE prewarm: dummy matmuls on zeros so the clock ramp starts
            # during the DMA head instead of on real data.
            warm = const.tile([128, 512], F16, tag="warm")
            nc.vector.memset(warm[:], 0.0)
            for r in range(10):
                pw = p3a.tile([128, 512], F32, tag="pa", name=f"warm{r}")
                nc.tensor.matmul(pw[:], warm[:, 0:128], warm[:],
                                 start=True, stop=True)
            wout = const.tile([128, 8], F32, tag="wout")
            nc.vector.tensor_copy(wout[:], pw[:, 0:8])
            nc.sync.dma_start(warm_d.ap(), wout[:])

            xhi = {}
            xlo = {}
            for n in range(NPC):
                xhi[n] = xpool.tile([128, 2, HW1], F16, tag=f"xhi{n}", name=f"xhi{n}")
                for k in range(K):
                    xlo[k, n] = xpool.tile([128, 2, HW1], E4,
                                           tag=f"xlo{k}{n}", name=f"xlo{k}{n}")

            # x DMAs: chunk-contiguous DRAM, sample-0 chunk0 spread over three
            # engines' queues so the first group's data lands fast.
            offh = 0
            offl = [k * nhi for k in range(K)]
            first = True
            for n in range(NPC):
                for p0, w in CHUNKS[n]:
                    span = 2 * 128 * w
                    src = xhi_d.ap()[offh : offh + span].rearrange(
                        "(kt p w) -> p kt w", kt=2, w=w)
                    nc.sync.dma_start(xhi[n][:, :, p0 : p0 + w], src)
                    offh += span
                    for k in range(K):
                        srcl = xlo_d.ap()[offl[k] : offl[k] + span].rearrange(
                            "(kt p w) -> p kt w", kt=2, w=w)
                        ek = nc.gpsimd if first else nc.sync
                        ek.dma_start(xlo[k, n][:, :, p0 : p0 + w], srcl)
                        offl[k] += span
                    first = False


            y1 = {}
            y2 = {}
            out01 = {}
            for n in range(NPC):
                y1[n] = ypool.tile([128, YSZ], E4, tag=f"y1_{n}", name=f"y1_{n}")
                y2[n] = ypool.tile([128, 2 * PT], E4, tag=f"y2_{n}", name=f"y2_{n}")
                out01[n] = opool.tile([128, HW1], E4, tag=f"o_{n}", name=f"o_{n}")

            # y1 pad + zero scratch block, set once on the idle Pool engine
            for n in range(NPC):
                v = y1[n][:].rearrange("p (h w) -> p h w", w=WP)
                nc.gpsimd.memset(v[:, 0:1, :], 0.0)
                nc.gpsimd.memset(v[:, 57:58, :], 0.0)
                nc.gpsimd.memset(v[:, 1:57, 0:1], 0.0)
                nc.gpsimd.memset(v[:, 1:57, 57:58], 0.0)

            def stage1(n):
                act = ST1_ACT[n]
                yb = y1[n][:]
                for g in range(4):
                    pp = p1pool.tile([128, 1024], F32, tag="p1")
                    # fp16 passes for both sub-tiles, then the fp8 DoubleRow
                    # passes back-to-back (entering DR mode right after a
                    # non-DR matmul stalls the PE ~135ns, so batch them)
                    for sub in range(2):
                        ps = slice(784 * g + PT * sub, 784 * g + PT * (sub + 1))
                        po = pp[:, 512 * sub : 512 * sub + PT]
                        nc.tensor.matmul(po, w16s(0), xhi[n][:, 0, ps],
                                         start=True, stop=False)
                        nc.tensor.matmul(po, w16s(128), xhi[n][:, 1, ps],
                                         start=False, stop=False)
                    for sub in range(2):
                        ps = slice(784 * g + PT * sub, 784 * g + PT * (sub + 1))
                        po = pp[:, 512 * sub : 512 * sub + PT]
                        for k in range(K):
                            nc.tensor.matmul(po, w8dr(k * 256),
                                             xlo[k, n][:, :, ps],
                                             start=False, stop=(k == K - 1),
                                             perf_mode=DR, skip_group_check=True)
                    pv = pp[:].rearrange("p (b c) -> p b c", b=2)[:, :, 0:PT]
                    ov = bass.AP(tensor=yb.tensor,
                                 offset=(14 * g + 1) * WP + 1,
                                 ap=[yb.ap[0], [WP, 14], [1, 56]])
                    if act:
                        nc.scalar.activation(ov, pv, AF.Sign,
                                             bias=bias[:, _B1A:_B1A + 1], scale=1.0)
                    else:
                        nc.vector.scalar_tensor_tensor(
                            ov, pv, bias[:, _B1V:_B1V + 1],
                            bias[:, _HALF:_HALF + 1].broadcast_to((128, 2, PT)),
                            op0=ALU.is_ge, op1=ALU.subtract)

            def stage2(n):
                enc_half = not ST1_ACT[n]
                pp = p1pool.tile([128, 1024], F32, tag="p1", name=f"p2_{n}")
                yb = y1[n][:]
                for ht in range(2):
                    po = pp[:, 512 * ht : 512 * ht + PT]
                    for p, (t0, t1) in enumerate(_PAIRS):
                        dy, dx = t0
                        off = (28 * ht + dy) * WP + dx
                        if t1 is None:
                            delta = ZOFF - off
                        elif t1[0] == dy:
                            delta = t1[1] - dx
                        else:
                            delta = (t1[0] - dy) * WP
                        rhs = bass.AP(tensor=yb.tensor, offset=off,
                                      ap=[yb.ap[0], [delta, 2], [2 * WP, 14], [2, 28]])
                        nc.tensor.matmul(po, w8dr(_WC2 + p * 256), rhs,
                                         start=(p == 0), stop=(p == 4),
                                         perf_mode=DR)
                pv = pp[:].rearrange("p (b c) -> p b c", b=2)[:, :, 0:PT]
                ov = y2[n][:].rearrange("p (b c) -> p b c", b=2)
                bcol = _B2H if enc_half else _B2F
                nc.scalar.activation(ov, pv, AF.Sign,
                                     bias=bias[:, bcol:bcol + 1], scale=1.0)



            def stage3(n):
                for ht in range(2):
                    hoff = 1568 * ht
                    for oc in range(4):
                        pa = p3a.tile([128, 512], F32, tag="pa", name=f"pa{oc}")
                        nc.tensor.matmul(pa[:, 0:PT], w8s(_W3C + oc * 128),
                                         y2[n][:, ht * PT : (ht + 1) * PT],
                                         start=True, stop=True)
                        pb = p3b.tile([128, 512], F32, tag="pb", name=f"pb{oc}")
                        # fp8 DoubleRow right after the fp8 conv3 matmul: the
                        # PE stalls ~140ns entering DR mode from an fp16 op
                        for k in range(K):
                            rhs = bass.AP(tensor=xlo[k, n][:].tensor, offset=hoff,
                                          ap=[xlo[k, n][:].ap[0], [HW1, 2],
                                              [112, 14], [2, 28]])
                            nc.tensor.matmul(pb[:, 0:PT],
                                             w8dr(_WSC + k * 1024 + oc * 256),
                                             rhs, start=(k == 0), stop=False,
                                             perf_mode=DR)
                        for kt in range(2):
                            rhs = bass.AP(tensor=xhi[n][:].tensor,
                                          offset=kt * HW1 + hoff,
                                          ap=[xhi[n][:].ap[0], [112, 14], [2, 28]])
                            nc.tensor.matmul(pb[:, 0:PT],
                                             w16s(256 + kt * 512 + oc * 128), rhs,
                                             start=False, stop=(kt == 1))
                        t = tpool.tile([128, PT], F32, tag="t", name=f"t{oc}")
                        nc.scalar.activation(t[:], pa[:, 0:PT], AF.Identity,
                                             bias=bias[:, _CC + oc:_CC + oc + 1],
                                             scale=bias[:, _QV + oc:_QV + oc + 1])
                        nc.vector.scalar_tensor_tensor(
                            out01[n][:, oc * 784 + ht * PT : oc * 784 + (ht + 1) * PT],
                            pb[:, 0:PT], -1.0, t[:], op0=ALU.mult, op1=ALU.is_le)

            for n in range(NPC):
                stage1(n)
                stage2(n)
                stage3(n)
                nc.gpsimd.dma_start(
                    out_d.ap()[n * 128 : (n + 1) * 128, 0:1568],
                    out01[n][:, 0:1568])
                (nc.sync if n == NPC - 1 else nc.gpsimd).dma_start(
                    out_d.ap()[n * 128 : (n + 1) * 128, 1568:HW1],
                    out01[n][:, 1568:HW1])

    nc.compile()
    return nc


def _prep_inputs(x, W1, W2, W3, Wsc, g1, b1, m1, v1, g2, b2, m2, v2,
                 g3, b3, m3, v3, gs, bs, ms, vs):
    f32 = np.float32

    def sgn(w):
        return np.where(w >= 0, 1.0, -1.0).astype(f32)

    def fold(g, b, m, v):
        s = (g / np.sqrt(v + EPS)).astype(f32)
        return s, (b - m * s).astype(f32)

    s1, c1 = fold(g1, b1, m1, v1)
    s2, c2 = fold(g2, b2, m2, v2)
    s3, c3 = fold(g3, b3, m3, v3)
    ss, cs = fold(gs, bs, ms, vs)

    B1 = sgn(W1[:, :, 0, 0])          # [128, 256]
    B2 = sgn(W2)                      # [128, 128, 3, 3]
    B3 = sgn(W3[:, :, 0, 0])          # [512, 128]
    Bs = sgn(Wsc[:, :, 0, 0])         # [512, 256]

    wts16 = np.zeros((128, _W16C), np.float16)
    for kt in range(2):
        wts16[:, kt * 128:(kt + 1) * 128] = (S * B1[:, kt * 128:(kt + 1) * 128].T)
        for oc in range(4):
            c0 = 256 + kt * 512 + oc * 128
            wts16[:, c0:c0 + 128] = \
                S * Bs[oc * 128:(oc + 1) * 128, kt * 128:(kt + 1) * 128].T

    wts8 = np.zeros((128, _W8C), E4NP)
    for k, sig in enumerate(SIGS):
        wk = f32(S / sig)
        for kt in range(2):
            c0 = k * 256 + kt * 128
            wts8[:, c0:c0 + 128] = (wk * B1[:, kt * 128:(kt + 1) * 128].T).astype(E4NP)
            for oc in range(4):
                c1_ = _WSC + k * 1024 + oc * 256 + kt * 128
                wts8[:, c1_:c1_ + 128] = \
                    (wk * Bs[oc * 128:(oc + 1) * 128,
                             kt * 128:(kt + 1) * 128].T).astype(E4NP)
    for p, (t0, t1) in enumerate(_PAIRS):
        if t1 is not None:
            for sl, tap in enumerate((t0, t1)):
                dy, dx = tap
                c0 = _WC2 + p * 256 + sl * 128
                wts8[:, c0:c0 + 128] = B2[:, :, dy, dx].T.astype(E4NP)
        else:
            dy, dx = t0
            c0 = _WC2 + 768 + (p - 3) * 128
            wts8[:, c0:c0 + 128] = B2[:, :, dy, dx].T.astype(E4NP)
    for oc in range(4):
        c0 = _W3C + oc * 128
        wts8[:, c0:c0 + 128] = B3[oc * 128:(oc + 1) * 128, :].T.astype(E4NP)

    biasv = np.zeros((128, 16), f32)
    biasv[:, _B1A] = f32(S) * (c1 / s1)
    biasv[:, _B1V] = -f32(S) * (c1 / s1)
    biasv[:, _B2F] = c2 / s2
    biasv[:, _B2H] = 0.5 * (c2 / s2)
    q = (s3 / ss).astype(f32)
    cc = ((c3 + cs) / ss).astype(f32)
    for oc in range(4):
        biasv[:, _QV + oc] = f32(S) * q[oc * 128:(oc + 1) * 128]
        biasv[:, _CC + oc] = f32(S) * cc[oc * 128:(oc + 1) * 128]
    biasv[:, _HALF] = 0.5

    xs = x.astype(f32)
    hi = xs.astype(np.float16)
    r = (xs - hi.astype(f32)).astype(f32)
    los = []
    for sig in SIGS:
        lo = (r * f32(sig)).astype(E4NP)
        los.append(lo)
        r = (r - lo.astype(f32) / f32(sig)).astype(f32)

    hi4 = hi.reshape(NB, 2, 128, HW1)
    lo4 = [lo.reshape(NB, 2, 128, HW1) for lo in los]

    def pack(xa):
        cores = []
        for c in range(NCORES):
            parts = []
            for n in range(NPC):
                g = xa[c * NPC + n]            # [2, 128, HW1]
                for p0, w in CHUNKS[n]:
                    parts.append(np.ascontiguousarray(g[:, :, p0:p0 + w]).reshape(-1))
            cores.append(np.concatenate(parts))
        return cores

    hic = pack(hi4)
    loc = [pack(l4) for l4 in lo4]
    xloc = [np.concatenate([loc[k][c] for k in range(K)]) for c in range(NCORES)]
    return hic, xloc, wts16, wts8, biasv


_NC_CACHE = []


def _assemble(res_results):
    outs = []
    for r in res_results:
        o = r["out"].astype(np.float32)              # [NPC*128, 3136] in {1,0}
        o = o.reshape(NPC, 128, 4, 2, 14, 28)
        o = o.transpose(0, 2, 1, 3, 4, 5).reshape(NPC, OUTP, HO, WO)
        outs.append(o)
    full = np.concatenate(outs, axis=0)
    return (2.0 * full - 1.0).astype(np.float32)


def make_in_maps(inputs):
    hic, xloc, wts16, wts8, biasv = _prep_inputs(**inputs)
    return [
        {"xhi": hic[c], "xlo": xloc[c], "w16": wts16, "w8": wts8, "bias": biasv}
        for c in range(NCORES)
    ]


def kernel(**inputs):
    inputs = {k: np.asarray(v) for k, v in inputs.items()}
    in_maps = make_in_maps(inputs)
    if not _NC_CACHE:
        _NC_CACHE.append(build_bass())
    nc = _NC_CACHE[0]
    res = run_bass_kernel_spmd(nc, in_maps, core_ids=list(range(NCORES)))
    return _assemble(res.results)


# revision 22
# speedup vs baseline: 1.0076x; 1.0076x over previous
"""Binarized ResNet Bottleneck block (dense_cnn) on 8 TRN2 NeuronCores.

Math: with inference BN folded to z*s + c (s > 0), binarize(htanh(bn(z))) ==
sign(z + c/s).  All conv weights binarize to +-1, so every conv is a GEMM with
pixels moving on the PE:

  y1 = sign(conv1(x)  + c1/s1)                     # 1x1, 256->128
  y2 = sign(conv2(y1) + c2/s2)                     # 3x3 stride 2, pad 1
  out = sign(q*conv3(y2) + convsc(x) + cc)         # q = s3/ss, cc=(c3+cs)/ss

x is split as x = hi + lo1/SIG1 + lo2/SIG2 with hi = fp16(x) and lo_k =
e4m3(r_k * SIG_k); the matching conv weights are +-S, +-S/SIG1, +-S/SIG2 (all
exact powers of two in their dtype), so each pass accumulates S*(w . r_k) into
the same f32 PSUM (residual ~2^-19 relative -> 0 output sign flips vs f32).
The fp8 passes pair the two 128-channel k-tiles into one DoubleRow matmul
(0.5 cy/col).  conv2 runs entirely in fp8: its 9 taps become 4 DoubleRow
pairs with constant address deltas + 1 pair whose second slot reads a
zeroed scratch block.  conv3 is a plain fp8 matmul with +-1 weights; its
per-channel scale q moves to a ScalarE Identity activation (scale/bias are
per-partition APs).

Sign ops are spread over two engines: ScalarE Sign gives +-1, VectorE
(is_ge, subtract 0.5) gives +-0.5 -- both exact in fp8, and the downstream
threshold columns absorb the per-sample encoding.  The final output is the
VectorE comparison (-sc <= q*conv3 + cc), i.e. {1,0}; the host maps it to
+-1.  PSUM: stage1/2 use [128,1024] two-bank pair tiles (one elementwise op
covers two matmul groups), stage3 double-buffers conv3/shortcut banks.
"""

import numpy as np
import ml_dtypes

import concourse.bass as bass
import concourse.tile as tile
from concourse import bacc, mybir
from concourse.bass_utils import run_bass_kernel_spmd

F16 = mybir.dt.float16
F32 = mybir.dt.float32
E4 = mybir.dt.float8e4
AF = mybir.ActivationFunctionType
ALU = mybir.AluOpType
DR = mybir.MatmulPerfMode.DoubleRow
E4NP = ml_dtypes.float8_e4m3  # TRN e4m3 is IEEE-style: max normal 240, exp 0xF = inf/nan

EPS = 1e-5
NB, CIN, H, W = 32, 256, 56, 56
PLANES, OUTP = 128, 512
NCORES = 8
NPC = NB // NCORES              # samples per core
HW1 = H * W                     # 3136
WP = W + 2                      # 58 padded row length
YSZ = WP * WP                   # 3364
HO = WO = 28
PT = 392                        # pixel tile (stage1: 7 rows of 56; stage3: 14x28)

S = 512.0                       # global PSUM scale
SIGS = (2.0 ** 16,)             # e4m3 residual part scales (scaled parts stay < 240)
K = len(SIGS)

# stage1 per-sample sign engine: True -> ScalarE Sign (+-1), False -> VectorE (+-0.5)
ST1_ACT = (True, True, False, False)

# per-sample input DMA pixel chunks
CHUNKS = {0: [(0, 784), (784, 784), (1568, 784), (2352, 784)]}
for _n in range(1, NPC):
    CHUNKS[_n] = [(0, 1568), (1568, 1568)]

# wts16 [128, 1280] fp16 columns: [0:256] conv1-hi (kt-major), [256:1280]
# shortcut-hi as [kt][oc][128].
_W16C = 1280
# wts8 columns: conv1-lo parts K x [kt][128], conv2 3 pairs x [2][128] then
# 3 singles x [128], conv3 [oc][128], shortcut-lo K x [kt][oc][128]
_WC2 = K * 256
_W3C = _WC2 + 1152
_WSC = _W3C + 512
_W8C = _WSC + K * 1024

# bias [128, 16] f32 columns
_B1A, _B1V, _B2F, _B2H, _QV, _CC, _HALF = 0, 1, 2, 3, 4, 8, 12

# conv2 tap pairs: (slot0 tap, slot1 tap or None, delta builder)
_PAIRS = [((0, 0), (0, 2)), ((1, 0), (1, 2)), ((2, 0), (2, 2)),
          ((0, 1), (2, 1)), ((1, 1), None)]


def build_bass():
    nc = bacc.Bacc("TRN2", target_bir_lowering=False, debug=False)
    nhi = NPC * 2 * 128 * HW1
    xhi_d = nc.dram_tensor("xhi", [nhi], F16, kind="ExternalInput")
    xlo_d = nc.dram_tensor("xlo", [K * nhi], E4, kind="ExternalInput")
    w16_d = nc.dram_tensor("w16", [128, _W16C], F16, kind="ExternalInput")
    w8_d = nc.dram_tensor("w8", [128, _W8C], E4, kind="ExternalInput")
    bias_d = nc.dram_tensor("bias", [128, 16], F32, kind="ExternalInput")
    out_d = nc.dram_tensor("out", [NPC * 128, HW1], E4, kind="ExternalOutput")
    warm_d = nc.dram_tensor("warm", [128, 8], F32, kind="ExternalOutput")

    with tile.TileContext(nc) as tc:
        import contextlib

        with contextlib.ExitStack() as ctx:
            const = ctx.enter_context(tc.tile_pool(name="const", bufs=1))
            xpool = ctx.enter_context(tc.tile_pool(name="x", bufs=1))
            ypool = ctx.enter_context(tc.tile_pool(name="y", bufs=1))
            opool = ctx.enter_context(tc.tile_pool(name="o", bufs=1))
            tpool = ctx.enter_context(tc.tile_pool(name="t", bufs=3))
            p1pool = ctx.enter_context(tc.tile_pool(name="p1", bufs=2, space="PSUM"))
            p3a = ctx.enter_context(tc.tile_pool(name="p3a", bufs=2, space="PSUM"))
            p3b = ctx.enter_context(tc.tile_pool(name="p3b", bufs=2, space="PSUM"))

            w16 = const.tile([128, _W16C], F16, tag="w16")
            # conv1-hi first so the first matmul isn't gated on the rest
            nc.scalar.dma_start(w16[:, 0:256], w16_d.ap()[:, 0:256])
            nc.scalar.dma_start(w16[:, 256:_W16C], w16_d.ap()[:, 256:_W16C])
            w8 = const.tile([128, _W8C], E4, tag="w8")
            nc.scalar.dma_start(w8[:, 0:_WC2], w8_d.ap()[:, 0:_WC2])
            nc.scalar.dma_start(w8[:, _WC2:_W8C], w8_d.ap()[:, _WC2:_W8C])
            bias = const.tile([128, 16], F32, tag="bias")
            nc.scalar.dma_start(bias[:], bias_d.ap())

            def w16s(col):
                return w16[:, col : col + 128]

            def w8dr(col):  # [128, 2, 128] DoubleRow stationary view
                return bass.AP(tensor=w8[:].tensor, offset=col,
                               ap=[w8[:].ap[0], [128, 2], [1, 128]])

            def w8s(col):
                return w8[:, col : col + 128]

            # PE prewarm: dummy matmuls on zeros so the clock ramp starts
            # during the DMA head instead of on real data.
            warm = const.tile([128, 512], F16, tag="warm")
            nc.vector.memset(warm[:], 0.0)
            for r in range(5):
                pw = p3a.tile([128, 512], F32, tag="pa", name=f"warm{r}")
                nc.tensor.matmul(pw[:], warm[:, 0:128], warm[:],
                                 start=True, stop=True)
            wout = const.tile([128, 8], F32, tag="wout")
            nc.vector.tensor_copy(wout[:], pw[:, 0:8])
            nc.sync.dma_start(warm_d.ap(), wout[:])

            xhi = {}
            xlo = {}
            for n in range(NPC):
                xhi[n] = xpool.tile([128, 2, HW1], F16, tag=f"xhi{n}", name=f"xhi{n}")
                for k in range(K):
                    xlo[k, n] = xpool.tile([128, 2, HW1], E4,
                                           tag=f"xlo{k}{n}", name=f"xlo{k}{n}")

            # x DMAs: chunk-contiguous DRAM, sample-0 chunk0 spread over three
            # engines' queues so the first group's data lands fast.
            offh = 0
            offl = [k * nhi for k in range(K)]
            first = True
            for n in range(NPC):
                for p0, w in CHUNKS[n]:
                    span = 2 * 128 * w
                    src = xhi_d.ap()[offh : offh + span].rearrange(
                        "(kt p w) -> p kt w", kt=2, w=w)
                    nc.sync.dma_start(xhi[n][:, :, p0 : p0 + w], src)
                    offh += span
                    for k in range(K):
                        srcl = xlo_d.ap()[offl[k] : offl[k] + span].rearrange(
                            "(kt p w) -> p kt w", kt=2, w=w)
                        ek = nc.gpsimd if first else nc.sync
                        ek.dma_start(xlo[k, n][:, :, p0 : p0 + w], srcl)
                        offl[k] += span
                    first = False


            y1 = {}
            y2 = {}
            out01 = {}
            for n in range(NPC):
                y1[n] = ypool.tile([128, YSZ], E4, tag=f"y1_{n}", name=f"y1_{n}")
                y2[n] = ypool.tile([128, 2 * PT], E4, tag=f"y2_{n}", name=f"y2_{n}")
                out01[n] = opool.tile([128, HW1], E4, tag=f"o_{n}", name=f"o_{n}")

            # y1 pad + zero scratch block, set once on the idle Pool engine
            for n in range(NPC):
                v = y1[n][:].rearrange("p (h w) -> p h w", w=WP)
                nc.gpsimd.memset(v[:, 0:1, :], 0.0)
                nc.gpsimd.memset(v[:, 57:58, :], 0.0)
                nc.gpsimd.memset(v[:, 1:57, 0:1], 0.0)
                nc.gpsimd.memset(v[:, 1:57, 57:58], 0.0)

            def stage1(n):
                act = ST1_ACT[n]
                yb = y1[n][:]
                for g in range(4):
                    pp = p1pool.tile([128, 1024], F32, tag="p1")
                    # fp16 passes for both sub-tiles, then the fp8 DoubleRow
                    # passes back-to-back (entering DR mode right after a
                    # non-DR matmul stalls the PE ~135ns, so batch them)
                    for sub in range(2):
                        ps = slice(784 * g + PT * sub, 784 * g + PT * (sub + 1))
                        po = pp[:, 512 * sub : 512 * sub + PT]
                        nc.tensor.matmul(po, w16s(0), xhi[n][:, 0, ps],
                                         start=True, stop=False)
                        nc.tensor.matmul(po, w16s(128), xhi[n][:, 1, ps],
                                         start=False, stop=False)
                    for sub in range(2):
                        ps = slice(784 * g + PT * sub, 784 * g + PT * (sub + 1))
                        po = pp[:, 512 * sub : 512 * sub + PT]
                        for k in range(K):
                            nc.tensor.matmul(po, w8dr(k * 256),
                                             xlo[k, n][:, :, ps],
                                             start=False, stop=(k == K - 1),
                                             perf_mode=DR, skip_group_check=True)
                    pv = pp[:].rearrange("p (b c) -> p b c", b=2)[:, :, 0:PT]
                    ov = bass.AP(tensor=yb.tensor,
                                 offset=(14 * g + 1) * WP + 1,
                                 ap=[yb.ap[0], [WP, 14], [1, 56]])
                    if act:
                        nc.scalar.activation(ov, pv, AF.Sign,
                                             bias=bias[:, _B1A:_B1A + 1], scale=1.0)
                    else:
                        nc.vector.scalar_tensor_tensor(
                            ov, pv, bias[:, _B1V:_B1V + 1],
                            bias[:, _HALF:_HALF + 1].broadcast_to((128, 2, PT)),
                            op0=ALU.is_ge, op1=ALU.subtract)

            def stage2(n):
                enc_half = not ST1_ACT[n]
                pp = p1pool.tile([128, 1024], F32, tag="p1", name=f"p2_{n}")
                yb = y1[n][:]
                for ht in range(2):
                    po = pp[:, 512 * ht : 512 * ht + PT]
                    for p, (t0, t1) in enumerate(_PAIRS):
                        dy, dx = t0
                        off = (28 * ht + dy) * WP + dx
                        if t1 is None:
                            delta = ZOFF - off
                        elif t1[0] == dy:
                            delta = t1[1] - dx
                        else:
                            delta = (t1[0] - dy) * WP
                        rhs = bass.AP(tensor=yb.tensor, offset=off,
                                      ap=[yb.ap[0], [delta, 2], [2 * WP, 14], [2, 28]])
                        nc.tensor.matmul(po, w8dr(_WC2 + p * 256), rhs,
                                         start=(p == 0), stop=(p == 4),
                                         perf_mode=DR)
                pv = pp[:].rearrange("p (b c) -> p b c", b=2)[:, :, 0:PT]
                ov = y2[n][:].rearrange("p (b c) -> p b c", b=2)
                bcol = _B2H if enc_half else _B2F
                nc.scalar.activation(ov, pv, AF.Sign,
                                     bias=bias[:, bcol:bcol + 1], scale=1.0)



            def stage3(n):
                for ht in range(2):
                    hoff = 1568 * ht
                    for oc in range(4):
                        pa = p3a.tile([128, 512], F32, tag="pa", name=f"pa{oc}")
                        nc.tensor.matmul(pa[:, 0:PT], w8s(_W3C + oc * 128),
                                         y2[n][:, ht * PT : (ht + 1) * PT],
                                         start=True, stop=True)
                        pb = p3b.tile([128, 512], F32, tag="pb", name=f"pb{oc}")
                        # fp8 DoubleRow right after the fp8 conv3 matmul: the
                        # PE stalls ~140ns entering DR mode from an fp16 op
                        for k in range(K):
                            rhs = bass.AP(tensor=xlo[k, n][:].tensor, offset=hoff,
                                          ap=[xlo[k, n][:].ap[0], [HW1, 2],
                                              [112, 14], [2, 28]])
                            nc.tensor.matmul(pb[:, 0:PT],
                                             w8dr(_WSC + k * 1024 + oc * 256),
                                             rhs, start=(k == 0), stop=False,
                                             perf_mode=DR)
                        for kt in range(2):
                            rhs = bass.AP(tensor=xhi[n][:].tensor,
                                          offset=kt * HW1 + hoff,
                                          ap=[xhi[n][:].ap[0], [112, 14], [2, 28]])
                            nc.tensor.matmul(pb[:, 0:PT],
                                             w16s(256 + kt * 512 + oc * 128), rhs,
                                             start=False, stop=(kt == 1))
                        t = tpool.tile([128, PT], F32, tag="t", name=f"t{oc}")
                        nc.scalar.activation(t[:], pa[:, 0:PT], AF.Identity,
                                             bias=bias[:, _CC + oc:_CC + oc + 1],
                                             scale=bias[:, _QV + oc:_QV + oc + 1])
                        nc.vector.scalar_tensor_tensor(
                            out01[n][:, oc * 784 + ht * PT : oc * 784 + (ht + 1) * PT],
                            pb[:, 0:PT], -1.0, t[:], op0=ALU.mult, op1=ALU.is_le)

            for n in range(NPC):
                stage1(n)
                stage2(n)
                stage3(n)
                nc.gpsimd.dma_start(
                    out_d.ap()[n * 128 : (n + 1) * 128, 0:1568],
                    out01[n][:, 0:1568])
                (nc.sync if n == NPC - 1 else nc.gpsimd).dma_start(
                    out_d.ap()[n * 128 : (n + 1) * 128, 1568:HW1],
                    out01[n][:, 1568:HW1])

    nc.compile()
    return nc


def _prep_inputs(x, W1, W2, W3, Wsc, g1, b1, m1, v1, g2, b2, m2, v2,
                 g3, b3, m3, v3, gs, bs, ms, vs):
    f32 = np.float32

    def sgn(w):
        return np.where(w >= 0, 1.0, -1.0).astype(f32)

    def fold(g, b, m, v):
        s = (g / np.sqrt(v + EPS)).astype(f32)
        return s, (b - m * s).astype(f32)

    s1, c1 = fold(g1, b1, m1, v1)
    s2, c2 = fold(g2, b2, m2, v2)
    s3, c3 = fold(g3, b3, m3, v3)
    ss, cs = fold(gs, bs, ms, vs)

    B1 = sgn(W1[:, :, 0, 0])          # [128, 256]
    B2 = sgn(W2)                      # [128, 128, 3, 3]
    B3 = sgn(W3[:, :, 0, 0])          # [512, 128]
    Bs = sgn(Wsc[:, :, 0, 0])         # [512, 256]

    wts16 = np.zeros((128, _W16C), np.float16)
    for kt in range(2):
        wts16[:, kt * 128:(kt + 1) * 128] = (S * B1[:, kt * 128:(kt + 1) * 128].T)
        for oc in range(4):
            c0 = 256 + kt * 512 + oc * 128
            wts16[:, c0:c0 + 128] = \
                S * Bs[oc * 128:(oc + 1) * 128, kt * 128:(kt + 1) * 128].T

    wts8 = np.zeros((128, _W8C), E4NP)
    for k, sig in enumerate(SIGS):
        wk = f32(S / sig)
        for kt in range(2):
            c0 = k * 256 + kt * 128
            wts8[:, c0:c0 + 128] = (wk * B1[:, kt * 128:(kt + 1) * 128].T).astype(E4NP)
            for oc in range(4):
                c1_ = _WSC + k * 1024 + oc * 256 + kt * 128
                wts8[:, c1_:c1_ + 128] = \
                    (wk * Bs[oc * 128:(oc + 1) * 128,
                             kt * 128:(kt + 1) * 128].T).astype(E4NP)
    for p, (t0, t1) in enumerate(_PAIRS):
        if t1 is not None:
            for sl, tap in enumerate((t0, t1)):
                dy, dx = tap
                c0 = _WC2 + p * 256 + sl * 128
                wts8[:, c0:c0 + 128] = B2[:, :, dy, dx].T.astype(E4NP)
        else:
            dy, dx = t0
            c0 = _WC2 + 768 + (p - 3) * 128
            wts8[:, c0:c0 + 128] = B2[:, :, dy, dx].T.astype(E4NP)
    for oc in range(4):
        c0 = _W3C + oc * 128
        wts8[:, c0:c0 + 128] = B3[oc * 128:(oc + 1) * 128, :].T.astype(E4NP)

    biasv = np.zeros((128, 16), f32)
    biasv[:, _B1A] = f32(S) * (c1 / s1)
    biasv[:, _B1V] = -f32(S) * (c1 / s1)
    biasv[:, _B2F] = c2 / s2
    biasv[:, _B2H] = 0.5 * (c2 / s2)
    q = (s3 / ss).astype(f32)
    cc = ((c3 + cs) / ss).astype(f32)
    for oc in range(4):
        biasv[:, _QV + oc] = f32(S) * q[oc * 128:(oc + 1) * 128]
        biasv[:, _CC + oc] = f32(S) * cc[oc * 128:(oc + 1) * 128]
    biasv[:, _HALF] = 0.5

    xs = x.astype(f32)
    hi = xs.astype(np.float16)
    r = (xs - hi.astype(f32)).astype(f32)
    los = []
    for sig in SIGS:
        lo = (r * f32(sig)).astype(E4NP)
        los.append(lo)
        r = (r - lo.astype(f32) / f32(sig)).astype(f32)

    hi4 = hi.reshape(NB, 2, 128, HW1)
    lo4 = [lo.reshape(NB, 2, 128, HW1) for lo in los]

    def pack(xa):
        cores = []
        for c in range(NCORES):
            parts = []
            for n in range(NPC):
                g = xa[c * NPC + n]            # [2, 128, HW1]
                for p0, w in CHUNKS[n]:
                    parts.append(np.ascontiguousarray(g[:, :, p0:p0 + w]).reshape(-1))
            cores.append(np.concatenate(parts))
        return cores

    hic = pack(hi4)
    loc = [pack(l4) for l4 in lo4]
    xloc = [np.concatenate([loc[k][c] for k in range(K)]) for c in range(NCORES)]
    return hic, xloc, wts16, wts8, biasv


_NC_CACHE = []


def _assemble(res_results):
    outs = []
    for r in res_results:
        o = r["out"].astype(np.float32)              # [NPC*128, 3136] in {1,0}
        o = o.reshape(NPC, 128, 4, 2, 14, 28)
        o = o.transpose(0, 2, 1, 3, 4, 5).reshape(NPC, OUTP, HO, WO)
        outs.append(o)
    full = np.concatenate(outs, axis=0)
    return (2.0 * full - 1.0).astype(np.float32)


def make_in_maps(inputs):
    hic, xloc, wts16, wts8, biasv = _prep_inputs(**inputs)
    return [
        {"xhi": hic[c], "xlo": xloc[c], "w16": wts16, "w8": wts8, "bias": biasv}
        for c in range(NCORES)
    ]


def kernel(**inputs):
    inputs = {k: np.asarray(v) for k, v in inputs.items()}
    in_maps = make_in_maps(inputs)
    if not _NC_CACHE:
        _NC_CACHE.append(build_bass())
    nc = _NC_CACHE[0]
    res = run_bass_kernel_spmd(nc, in_maps, core_ids=list(range(NCORES)))
    return _assemble(res.results)
